# revision 3
# baseline (speedup 1.0000x reference)
"""GRU-cell-variant kernel for Trainium2, data-parallel over batch on 8 cores.

Reference (per batch row b, hidden size H=1024):
    gates = sigmoid(x @ W_ih + b_ih + h @ W_hh + b_hh)   # [B, 2H]
    z, r  = gates[:, :H], gates[:, H:]
    cand  = tanh(x @ W_c + b_c + r * (h @ W_hc + b_hc))
    out   = (1 - z) * h + z * cand

Design:
  - 8-way batch shard (1024 rows/core), weights replicated. No collectives.
  - Everything on-chip is computed TRANSPOSED: out.T[o, b]. Weight tiles
    [k, o] load naturally as the stationary operand, host-pre-transposed
    x.T / h.T serve as the moving operand, and all biases are per-partition
    (free bias-add on the ACT engine).
  - Mixed fp8/fp16 matmuls: the z/r gate matmuls and h@W_hc run as
    fp8-e4m3 DoubleRow (2 contraction chunks per PE pass -> ~1.5-1.8x the
    bf16 matmul rate); x@W_c stays fp16 because its quantization error
    feeds tanh unattenuated (measured L2 budget: all-fp8 2.05e-2 vs this
    mix 1.45e-2 against the 2e-2 gate).
  - fp8 operands are pre-scaled on the host (x,h by 2^4; W by 2^8) to stay
    clear of e4m3 subnormals; the combined 2^-12 descale folds into the
    scale parameter of the existing sigmoid/tanh activation ops. The fp16
    W_c is pre-scaled by 2^12 so both candidate partial sums share one
    scale. fp32 PSUM accumulation throughout; h-residual path in fp16.
  - Host packs weights/activations into the exact SBUF layouts so every DMA
    is a dense 2D/3D copy.
"""

import numpy as np
import ml_dtypes

import concourse.bass as bass
import concourse.mybir as mybir
import concourse.tile as tile
from concourse import bacc
from concourse.bass_utils import run_bass_kernel_spmd

N_CORES = 8
B = 8192
H = 1024
BL = B // N_CORES  # batch rows per core
P = 128
KC = H // P  # 8 contraction chunks of 128 per 1024-wide operand
NJ = H // P  # 8 hidden-dim tiles
NB = BL // 512  # 2 moving halves of 512 batch columns

F8 = mybir.dt.float8e4
F16 = mybir.dt.float16
F32 = mybir.dt.float32
AF = mybir.ActivationFunctionType
ALU = mybir.AluOpType
DR = mybir.MatmulPerfMode.DoubleRow

ASCALE = 16.0  # activation fp8 pre-scale
WSCALE = 256.0  # weight fp8 pre-scale
SCALE_INV = 1.0 / (ASCALE * WSCALE)  # descale folded into ACT ops

_CACHE = {}


def _build_program():
    nc = bacc.Bacc(
        "TRN2",
        target_bir_lowering=False,
        debug=False,
        enable_asserts=False,
        num_devices=N_CORES,
    )

    # DRAM inputs, already packed on the host into SBUF-friendly layouts.
    # x8/h8:  [p, kc*BL + b] = x[b, kc*128 + p] * 16   (fp8 e4m3)
    # x16:    same layout, unscaled fp16 (W_c matmul operand)
    # h16:    same layout, unscaled fp16 (residual path)
    # Wg:     [p, t*2048 + kc*128 + jj] = Wg_full[kc*128+p, t*128+jj]*256 (fp8)
    #          t in [0,16): gate output tile; kc in [0,16): contraction over [x;h]
    # Whc:    [p, j*1024 + kc*128 + jj] = W_hc[kc*128+p, j*128+jj]*256  (fp8)
    # Wc:     same layout, W_c * 4096  (fp16)
    # bg:     [p, t] = (b_ih+b_hh)[t*128+p]; bc analogous; bhc pre-scaled 4096.
    x8 = nc.dram_tensor("x8", [P, KC * BL], F8, kind="ExternalInput").ap()
    h8 = nc.dram_tensor("h8", [P, KC * BL], F8, kind="ExternalInput").ap()
    x16 = nc.dram_tensor("x16", [P, KC * BL], F16, kind="ExternalInput").ap()
    h16 = nc.dram_tensor("h16", [P, NJ * BL], F16, kind="ExternalInput").ap()
    Wg = nc.dram_tensor("Wg", [P, 16 * 2048], F8, kind="ExternalInput").ap()
    Wc = nc.dram_tensor("Wc", [P, NJ * H], F16, kind="ExternalInput").ap()
    Whc = nc.dram_tensor("Whc", [P, NJ * H], F8, kind="ExternalInput").ap()
    bg = nc.dram_tensor("bg", [P, 16], F32, kind="ExternalInput").ap()
    bc = nc.dram_tensor("bc", [P, NJ], F32, kind="ExternalInput").ap()
    bhc = nc.dram_tensor("bhc", [P, NJ], F32, kind="ExternalInput").ap()
    outT = nc.dram_tensor("outT", [P, NJ * BL], F32, kind="ExternalOutput").ap()

    with tile.TileContext(nc) as tc:
        with (
            tc.tile_pool(name="const", bufs=1) as cpool,
            tc.tile_pool(name="wg", bufs=4) as wgpool,
            tc.tile_pool(name="whc", bufs=4) as whcpool,
            tc.tile_pool(name="wc", bufs=4) as wcpool,
            tc.tile_pool(name="psum", bufs=8, space="PSUM") as ppool,
            tc.tile_pool(name="gates", bufs=6) as gpool,
            tc.tile_pool(name="work", bufs=10) as wpool,
        ):
            bg_sb = cpool.tile([P, 16], F32, tag="bg")
            bc_sb = cpool.tile([P, NJ], F32, tag="bc")
            bhc_sb = cpool.tile([P, NJ], F32, tag="bhc")

            # Resident activations. fp8 copies feed the DoubleRow matmuls,
            # x16 feeds the fp16 W_c matmul, h16 feeds the residual blend.
            x8_sb = cpool.tile([P, KC * BL], F8, tag="x8")
            h8_sb = cpool.tile([P, KC * BL], F8, tag="h8")
            x16_sb = cpool.tile([P, KC * BL], F16, tag="x16")
            h16_sb = cpool.tile([P, NJ * BL], F16, tag="h16")

            # 3D views [p, kc, b] for DoubleRow rhs slices and chunk loads
            x8s3 = x8_sb[:].rearrange("p (kc b) -> p kc b", kc=KC)
            x8d3 = x8.rearrange("p (kc b) -> p kc b", kc=KC)
            h8s3 = h8_sb[:].rearrange("p (kc b) -> p kc b", kc=KC)
            h8d3 = h8.rearrange("p (kc b) -> p kc b", kc=KC)
            x16s3 = x16_sb[:].rearrange("p (kc b) -> p kc b", kc=KC)
            x16d3 = x16.rearrange("p (kc b) -> p kc b", kc=KC)

            def gate_matmuls(psum, w3, b0, cs=range(2 * KC // 2)):
                # accumulate over [x;h]: 8 DoubleRow passes of K=256 each;
                # pair c<4 reads x8, c>=4 reads h8
                for c in cs:
                    src = x8s3 if c < KC // 2 else h8s3
                    k0 = (2 * c) % KC
                    nc.tensor.matmul(
                        psum[:],
                        lhsT=w3[:, 2 * c : 2 * c + 2, :],
                        rhs=src[:, k0 : k0 + 2, b0 : b0 + 512],
                        start=(c == 0),
                        stop=(c == KC - 1),
                        perf_mode=DR,
                    )

            def hc_matmuls(psum, w3, b0):
                for c in range(KC // 2):
                    nc.tensor.matmul(
                        psum[:],
                        lhsT=w3[:, 2 * c : 2 * c + 2, :],
                        rhs=h8s3[:, 2 * c : 2 * c + 2, b0 : b0 + 512],
                        start=(c == 0),
                        stop=(c == KC // 2 - 1),
                        perf_mode=DR,
                    )

            def c_matmuls(psum, w_sb, b0):
                for kc in range(KC):
                    off = kc * BL + b0
                    nc.tensor.matmul(
                        psum[:],
                        lhsT=w_sb[:, kc * P : (kc + 1) * P],
                        rhs=x16_sb[:, off : off + 512],
                        start=(kc == 0),
                        stop=(kc == KC - 1),
                    )

            for j in range(NJ):
                wz = wgpool.tile([P, 2048], F8, tag="wg")
                wr = wgpool.tile([P, 2048], F8, tag="wg")
                whc_w = whcpool.tile([P, H], F8, tag="whc")
                wc_w = wcpool.tile([P, H], F16, tag="wc")
                wz3 = wz[:].rearrange("p (kc m) -> p kc m", kc=16)
                wr3 = wr[:].rearrange("p (kc m) -> p kc m", kc=16)
                whc3 = whc_w[:].rearrange("p (kc m) -> p kc m", kc=KC)
                if j == 0:
                    # Cold-start feed across BOTH HWDGE rings so the issue
                    # streams run in parallel. sync ring: fp8 activations
                    # (first matmul operands) then x16 halves. ACT ring:
                    # j0 weights + constants + h16 j0.
                    nc.sync.dma_start(
                        x8s3[:, :, 0:512], x8d3[:, :, 0:512]
                    )  # x8 b0
                    nc.sync.dma_start(
                        h8s3[:, :, 0:512], h8d3[:, :, 0:512]
                    )  # h8 b0
                    nc.sync.dma_start(
                        x16s3[:, 0:4, 0:512], x16d3[:, 0:4, 0:512]
                    )  # x16 b0 lo
                    nc.sync.dma_start(
                        x16s3[:, 4:8, 0:512], x16d3[:, 4:8, 0:512]
                    )  # x16 b0 hi
                    nc.sync.dma_start(
                        x8s3[:, :, 512:1024], x8d3[:, :, 512:1024]
                    )  # x8 b1
                    nc.sync.dma_start(
                        h8s3[:, :, 512:1024], h8d3[:, :, 512:1024]
                    )  # h8 b1

                    nc.scalar.dma_start(wz[:, 0:1024], Wg[:, 0:1024])
                    nc.scalar.dma_start(wz[:, 1024:2048], Wg[:, 1024:2048])
                    nc.scalar.dma_start(bg_sb[:], bg[:])
                    nc.scalar.dma_start(
                        wr[:, 0:1024], Wg[:, NJ * 2048 : NJ * 2048 + 1024]
                    )
                    nc.scalar.dma_start(
                        wr[:, 1024:2048], Wg[:, NJ * 2048 + 1024 : NJ * 2048 + 2048]
                    )
                    nc.scalar.dma_start(bc_sb[:], bc[:])
                    nc.scalar.dma_start(bhc_sb[:], bhc[:])
                    nc.scalar.dma_start(whc_w[:], Whc[:, 0:H])
                    nc.scalar.dma_start(wc_w[:], Wc[:, 0:H])
                    nc.scalar.dma_start(
                        x16s3[:, 0:4, 512:1024], x16d3[:, 0:4, 512:1024]
                    )  # x16 b1 lo
                    nc.scalar.dma_start(
                        x16s3[:, 4:8, 512:1024], x16d3[:, 4:8, 512:1024]
                    )  # x16 b1 hi
                elif j == 1:
                    # split j=1 weights across the two rings
                    nc.sync.dma_start(wz[:], Wg[:, 1 * 2048 : 2 * 2048])
                    nc.scalar.dma_start(wr[:], Wg[:, (NJ + 1) * 2048 : (NJ + 2) * 2048])
                    nc.sync.dma_start(whc_w[:], Whc[:, H : 2 * H])
                    nc.scalar.dma_start(wc_w[:], Wc[:, H : 2 * H])
                else:
                    nc.sync.dma_start(wz[:], Wg[:, j * 2048 : (j + 1) * 2048])
                    nc.sync.dma_start(
                        wr[:], Wg[:, (NJ + j) * 2048 : (NJ + j + 1) * 2048]
                    )
                    nc.sync.dma_start(whc_w[:], Whc[:, j * H : (j + 1) * H])
                    nc.sync.dma_start(wc_w[:], Wc[:, j * H : (j + 1) * H])
                # residual-path h (fp16) rides the second HWDGE ring (ACT)
                nc.scalar.dma_start(
                    h16_sb[:, j * BL : (j + 1) * BL], h16[:, j * BL : (j + 1) * BL]
                )

                for b in range(NB):
                    b0 = b * 512
                    hoff = j * BL + b0  # slice of hidden tile j in [p, j*BL+b] layout

                    pz = ppool.tile([P, 512], F32, tag="ps")
                    if j == 0 and b == 0:
                        # cold start: interleave z/r accumulation in 2-pair
                        # blocks matching the DMA arrival order (x8 b0 first,
                        # then h8 b0; PE executes its stream in order)
                        pr = ppool.tile([P, 512], F32, tag="ps")
                        for blk in range(2):
                            gate_matmuls(pz, wz3, b0, cs=range(2 * blk, 2 * blk + 2))
                            gate_matmuls(pr, wr3, b0, cs=range(2 * blk, 2 * blk + 2))
                        for blk in range(2, 4):
                            gate_matmuls(pz, wz3, b0, cs=range(2 * blk, 2 * blk + 2))
                            gate_matmuls(pr, wr3, b0, cs=range(2 * blk, 2 * blk + 2))
                    else:
                        gate_matmuls(pz, wz3, b0)
                        pr = None
                    z_sb = gpool.tile([P, 512], F32, tag="g")
                    nc.scalar.activation(
                        z_sb[:], pz[:], AF.Sigmoid,
                        bias=bg_sb[:, j : j + 1], scale=SCALE_INV,
                    )
                    # zh = (z - 1) * h, computed off the critical path so the
                    # post-tanh chain is only mul + subtract
                    zh = wpool.tile([P, 512], F32, tag="w")
                    nc.vector.scalar_tensor_tensor(
                        zh[:], z_sb[:], 1.0, h16_sb[:, hoff : hoff + 512],
                        ALU.subtract, ALU.mult,
                    )

                    if pr is None:
                        pr = ppool.tile([P, 512], F32, tag="ps")
                        gate_matmuls(pr, wr3, b0)
                    r_sb = gpool.tile([P, 512], F32, tag="g")
                    nc.scalar.activation(
                        r_sb[:], pr[:], AF.Sigmoid,
                        bias=bg_sb[:, NJ + j : NJ + j + 1], scale=SCALE_INV,
                    )

                    ph = ppool.tile([P, 512], F32, tag="ps")
                    hc_matmuls(ph, whc3, b0)
                    px = ppool.tile([P, 512], F32, tag="ps")
                    c_matmuls(px, wc_w, b0)

                    # candidate + output blend. ph/px/bhc share the 4096x
                    # scale; the tanh activation applies the descale.
                    rh = wpool.tile([P, 512], F32, tag="w")
                    nc.vector.scalar_tensor_tensor(
                        rh[:], ph[:], bhc_sb[:, j : j + 1], r_sb[:],
                        ALU.add, ALU.mult,
                    )
                    s = wpool.tile([P, 512], F32, tag="w")
                    nc.vector.tensor_add(s[:], px[:], rh[:])
                    cand = wpool.tile([P, 512], F32, tag="w")
                    nc.scalar.activation(
                        cand[:], s[:], AF.Tanh,
                        bias=bc_sb[:, j : j + 1], scale=SCALE_INV,
                    )
                    # out = z*cand - (z-1)*h
                    m = wpool.tile([P, 512], F32, tag="w")
                    nc.vector.tensor_mul(m[:], z_sb[:], cand[:])
                    o_sb = wpool.tile([P, 512], F32, tag="w")
                    nc.vector.tensor_sub(o_sb[:], m[:], zh[:])
                    nc.scalar.dma_start(outT[:, hoff : hoff + 512], o_sb[:])

    nc.compile()
    return nc


def _pack_weights(W_ih, b_ih, W_hh, b_hh, W_c, b_c, W_hc, b_hc):
    f8 = ml_dtypes.float8_e4m3
    Wg_full = np.concatenate([W_ih, W_hh], axis=0)  # [2H, 2H] = [k, o]
    WgH = np.ascontiguousarray(
        Wg_full.reshape(16, P, 16, P).transpose(1, 2, 0, 3).reshape(P, 16 * 2048)
        * WSCALE
    ).astype(f8)
    WcH = np.ascontiguousarray(
        W_c.reshape(KC, P, NJ, P).transpose(1, 2, 0, 3).reshape(P, NJ * H)
        * (ASCALE * WSCALE)
    ).astype(np.float16)
    WhcH = np.ascontiguousarray(
        W_hc.reshape(KC, P, NJ, P).transpose(1, 2, 0, 3).reshape(P, NJ * H) * WSCALE
    ).astype(f8)
    bgH = np.ascontiguousarray((b_ih + b_hh).reshape(16, P).T).astype(np.float32)
    bcH = np.ascontiguousarray(b_c.reshape(NJ, P).T).astype(np.float32)
    bhcH = np.ascontiguousarray(b_hc.reshape(NJ, P).T * (ASCALE * WSCALE)).astype(
        np.float32
    )
    return WgH, WcH, WhcH, bgH, bcH, bhcH


def _pack_acts(a, dtype, scale=1.0):
    # [BL, H] -> [p, kc*BL + b] with a[b, kc*128+p]
    out = a.T.reshape(KC, P, BL).transpose(1, 0, 2).reshape(P, KC * BL)
    if scale != 1.0:
        out = out * scale
    return np.ascontiguousarray(out).astype(dtype)


def kernel(input, hx, W_ih, b_ih, W_hh, b_hh, W_c, b_c, W_hc, b_hc):
    input = np.asarray(input, np.float32)
    hx = np.asarray(hx, np.float32)
    if "nc" not in _CACHE:
        _CACHE["nc"] = _build_program()
    nc = _CACHE["nc"]

    WgH, WcH, WhcH, bgH, bcH, bhcH = _pack_weights(
        np.asarray(W_ih, np.float32), np.asarray(b_ih, np.float32),
        np.asarray(W_hh, np.float32), np.asarray(b_hh, np.float32),
        np.asarray(W_c, np.float32), np.asarray(b_c, np.float32),
        np.asarray(W_hc, np.float32), np.asarray(b_hc, np.float32),
    )

    f8 = ml_dtypes.float8_e4m3
    in_maps = []
    for i in range(N_CORES):
        xs = input[i * BL : (i + 1) * BL]
        hs = hx[i * BL : (i + 1) * BL]
        in_maps.append(
            {
                "x8": _pack_acts(xs, f8, ASCALE),
                "h8": _pack_acts(hs, f8, ASCALE),
                "x16": _pack_acts(xs, np.float16),
                "h16": _pack_acts(hs, np.float16),
                "Wg": WgH,
                "Wc": WcH,
                "Whc": WhcH,
                "bg": bgH,
                "bc": bcH,
                "bhc": bhcH,
            }
        )

    res = run_bass_kernel_spmd(nc, in_maps, core_ids=list(range(N_CORES)))
    out = np.empty((B, H), np.float32)
    for i, r in enumerate(res.results):
        o = r["outT"].reshape(P, NJ, BL).transpose(2, 1, 0).reshape(BL, H)
        out[i * BL : (i + 1) * BL] = o
    return out
